# revision 21
# baseline (speedup 1.0000x reference)
"""DeepseekV3 MLA flash-attention prefill kernel for 8 Trainium2 NeuronCores.

Sharding (SPMD, one program for all 8 cores):
  Stage A (sequence-parallel): core c owns 256 seq rows. Weights stream in
    two waves (kv cols first, then q cols); each arriving 128-row weight
    tile is consumed into PSUM-resident accumulators for ALL output chunks,
    so the kv AllGather triggers as soon as the kv wave drains (~15us) and
    the q AllGather (split into two 6-chunk halves) right after the q wave.
  Stage B (head-parallel): core c owns heads {2c, 2c+1}. K^T/V from the kv
    gather; q projections accumulate the two gathered qa halves (so the
    second q AllGather hides under the first half's projection work).
    Causal attention runs in (k, q) layout, no max-subtraction, fully-masked
    k-blocks skipped, diagonal blocks masked by a vector mask-add (softmax
    scale pre-folded into Wqb host-side).
  Output: per-panel partial Wo products (only this core's 2 head-rows of Wo)
    are ReduceScattered over seq in two groups; the first RS overlaps the
    attention of the later (heavier) panels. Host reassembles rows.
"""

import sys

if '/opt/trn_rl_repo' not in sys.path:
    sys.path.insert(0, '/opt/trn_rl_repo')

import numpy as np
import ml_dtypes

import concourse.bass as bass
import concourse.mybir as mybir
import concourse.tile as tile
from concourse import bacc
from concourse.bass_utils import run_bass_kernel_spmd

f32 = mybir.dt.float32
f32r = mybir.dt.float32r
bf16 = mybir.dt.bfloat16
i32 = mybir.dt.int32
AF = mybir.ActivationFunctionType
ALU = mybir.AluOpType

NC_ = 8            # cores
S = 2048           # sequence
HID = 2048
QLR = 1536         # q lora rank
KVLR = 512         # kv lora rank
ROPE = 64
NOPE = 128
VD = 128
NH = 16
HPC = NH // NC_    # heads per core = 2
SL = S // NC_      # rows per core = 256
PANEL = 512        # q panel width
NPANEL = S // PANEL
NKB = S // 128     # 16 k blocks
QCH = QLR // 128   # 12
QHALF = QCH // 2   # 6
KCH = KVLR // 128  # 4
HCH = HID // 128   # 16
KVW = KVLR + ROPE  # 576 = kv wave width
THETA = 10000.0
SM_SCALE = float((NOPE + ROPE) ** -0.5)
PI = float(np.pi)
NEG = -1e30

DT = bf16
RS_DT = bf16       # dtype of the output ReduceScatter

_CACHE = {}


def _range_reduce_sin(nc, pool, src_ap, P, W, bias, name, res_pool=None, tagw=""):
    """sin(src + bias) with range reduction to [-pi, pi]. src may be PSUM."""
    t0 = pool.tile([P, W], f32, name=f"{name}_t0", tag=f"rr0{tagw}", bufs=1)
    ti = pool.tile([P, W], i32, name=f"{name}_ti", tag=f"rr1{tagw}", bufs=1)
    tf = pool.tile([P, W], f32, name=f"{name}_tf", tag=f"rr2{tagw}", bufs=1)
    arg = pool.tile([P, W], f32, name=f"{name}_arg", tag=f"rr3{tagw}", bufs=1)
    res = (res_pool or pool).tile([P, W], f32 if res_pool is None else bf16,
                                  name=f"{name}_sin", tag=f"res_{name}", bufs=1)
    nc.vector.tensor_scalar(out=t0[:], in0=src_ap, scalar1=bias, scalar2=None, op0=ALU.add)
    nc.vector.tensor_scalar(out=tf[:], in0=t0[:], scalar1=1.0 / (2 * PI), scalar2=None, op0=ALU.mult)
    nc.vector.tensor_copy(ti[:], tf[:])
    nc.vector.tensor_copy(tf[:], ti[:])
    nc.vector.scalar_tensor_tensor(out=arg[:], in0=tf[:], scalar=-2 * PI, in1=t0[:], op0=ALU.mult, op1=ALU.add)
    nc.scalar.activation(res[:], arg[:], AF.Sin)
    return res


def build_program(dt):
    nc = bacc.Bacc("TRN2", target_bir_lowering=False, debug=False, num_devices=NC_)

    def din(name, shape):
        return nc.dram_tensor(name, shape, dt, kind="ExternalInput")

    # ---- external I/O (per-core data) ----
    x_t = din("x_t", [HID, SL])                  # X rows, transposed (hid-major)
    pos = nc.dram_tensor("pos", [1, SL], f32, kind="ExternalInput")
    pos_all = nc.dram_tensor("pos_all", [1, S], f32, kind="ExternalInput")
    wa_kv = din("wa_kv", [HID, KVW])             # [Wkva(kv) | Wkva(pe, deint)]
    wa_q = din("wa_q", [HID, QLR])               # Wqa
    wqb = din("wqb", [QLR, HPC * 256])           # [nope|pe_d|rot]*SM_SCALE per head
    wkvb_k = din("wkvb_k", [KVLR, HPC * NOPE])
    wkvb_v = din("wkvb_v", [KVLR, HPC * VD])
    wo_h = din("wo_h", [HPC * VD, HID])          # Wo rows for this core's heads
    mask_in = din("mask", [128, 4 * PANEL])      # diag masks j=0..3 (0 / -1e30)
    ones_col = din("ones_col", [128, 1])
    ones_row = nc.dram_tensor("ones_row", [1, 128], f32, kind="ExternalInput")
    invf_col = nc.dram_tensor("invf_col", [ROPE, 1], f32, kind="ExternalInput")
    out_loc = nc.dram_tensor("out_loc", [2 * 128, HID], f32, kind="ExternalOutput")

    with tile.TileContext(nc) as tc:
        with tc.tile_pool(name="dram", bufs=1, space="DRAM") as dpool, \
             tc.tile_pool(name="persist", bufs=1) as rp:
            ag_in_kv = dpool.tile([KVW, SL], dt)
            ag_out_kv = dpool.tile([NC_ * KVW, SL], dt, addr_space="Shared")
            ag_in_qa = dpool.tile([QHALF * 128, SL], dt)
            ag_out_qa = dpool.tile([NC_ * QHALF * 128, SL], dt, addr_space="Shared")
            ag_in_qb = dpool.tile([QHALF * 128, SL], dt)
            ag_out_qb = dpool.tile([NC_ * QHALF * 128, SL], dt, addr_space="Shared")
            rs_in = [dpool.tile([2 * PANEL, HID], RS_DT, name=f"rs_in{g}") for g in range(2)]
            rs_out = [dpool.tile([2 * PANEL // NC_, HID], RS_DT, name=f"rs_out{g}")
                      for g in range(2)]

            # ---- constants ----
            ocol = rp.tile([128, 1], dt)
            orow = rp.tile([1, 128], f32r)
            invc_t = rp.tile([ROPE, 1], f32)
            nc.sync.dma_start(out=ocol[:], in_=ones_col[:])
            nc.sync.dma_start(out=orow[:], in_=ones_row[:].bitcast(f32r))
            nc.sync.dma_start(out=invc_t[:], in_=invf_col[:])

            # ---- stage B weights prefetch (gpsimd queue so the sync queue
            #      stays dedicated to the stage-A stream) ----
            mask_sb = rp.tile([128, 4 * PANEL], dt, name="mask_sb")
            nc.gpsimd.dma_start(out=mask_sb[:], in_=mask_in[:])
            wqb_t = []
            for l in range(QCH):
                t = rp.tile([128, HPC * 256], dt, name=f"wqb_t{l}")
                nc.gpsimd.dma_start(out=t[:], in_=wqb[128 * l:128 * (l + 1), :])
                wqb_t.append(t)
            wkk_t = []
            wkv_t = []
            for l in range(KCH):
                t = rp.tile([128, HPC * NOPE], dt, name=f"wkk_t{l}")
                nc.gpsimd.dma_start(out=t[:], in_=wkvb_k[128 * l:128 * (l + 1), :])
                wkk_t.append(t)
                t2 = rp.tile([128, HPC * VD], dt, name=f"wkv_t{l}")
                nc.gpsimd.dma_start(out=t2[:], in_=wkvb_v[128 * l:128 * (l + 1), :])
                wkv_t.append(t2)
            wo_sb = []
            for h in range(HPC):
                t = rp.tile([128, HID], dt, name=f"wo_sb{h}")
                nc.gpsimd.dma_start(out=t[:], in_=wo_h[VD * h:VD * (h + 1), :])
                wo_sb.append(t)

            # rope tables (results persist into stage B)
            sin_all = None
            cos_all = None

            # ================= Stage A =================
            with tc.tile_pool(name="sa_in", bufs=1) as sap, \
                 tc.tile_pool(name="sa_tmp", bufs=2) as tp, \
                 tc.tile_pool(name="sa_ps", bufs=6, space="PSUM") as accp, \
                 tc.tile_pool(name="sa_ps1", bufs=1, space="PSUM") as pp1:

                # stage A input streams (sync queue, in arrival-priority order)
                xts = []
                for k in range(HCH):
                    xt = sap.tile([128, SL], dt, name=f"xt{k}")
                    nc.sync.dma_start(out=xt[:], in_=x_t[128 * k:128 * (k + 1), :])
                    xts.append(xt)
                wkv_tiles = []
                for k in range(HCH):
                    t = sap.tile([128, KVW], dt, name=f"wakv{k}")
                    nc.sync.dma_start(out=t[:], in_=wa_kv[128 * k:128 * (k + 1), :])
                    wkv_tiles.append(t)
                wq_tiles = []
                for k in range(HCH):
                    t = sap.tile([128, QLR], dt, name=f"waq{k}")
                    nc.sync.dma_start(out=t[:], in_=wa_q[128 * k:128 * (k + 1), :])
                    wq_tiles.append(t)

                pos_all_t = tp.tile([1, S], f32r, name="pos_all_t", tag="posa", bufs=1)
                pos_t = tp.tile([1, SL], f32r, name="pos_t", tag="poso", bufs=1)
                nc.sync.dma_start(out=pos_all_t[:], in_=pos_all[:].bitcast(f32r))
                nc.sync.dma_start(out=pos_t[:], in_=pos[:].bitcast(f32r))
                emb_all = tp.tile([ROPE, S], f32, name="emb_all", tag="emba", bufs=1)

                # rope angle tables via K=1 outer products (one PSUM bank
                # per accumulation group -- matmul start zeroes a whole bank)
                for j in range(S // SL):
                    tb = accp.tile([128, SL], f32, name=f"tb_all{j}", tag="acc", bufs=6)
                    nc.tensor.matmul(tb[0:ROPE, :], orow[0:1, 0:ROPE],
                                     pos_all_t[:, SL * j:SL * (j + 1)], start=True, stop=True)
                    nc.vector.tensor_scalar(out=emb_all[:, SL * j:SL * (j + 1)],
                                            in0=tb[0:ROPE, :], scalar1=invc_t[:],
                                            scalar2=None, op0=ALU.mult)
                tb_own = accp.tile([128, SL], f32, name="tb_own", tag="acc", bufs=6)
                nc.tensor.matmul(tb_own[0:ROPE, 0:SL], orow[0:1, 0:ROPE], pos_t[:],
                                 start=True, stop=True)
                emb_own = tp.tile([ROPE, SL], f32, name="emb_own", tag="emb_own", bufs=1)
                nc.vector.tensor_scalar(out=emb_own[:], in0=tb_own[0:ROPE, 0:SL],
                                        scalar1=invc_t[:], scalar2=None, op0=ALU.mult)

                sin_all = _range_reduce_sin(nc, tp, emb_all[:], ROPE, S, 0.0, "sa",
                                            res_pool=rp, tagw="w")
                cos_all = _range_reduce_sin(nc, tp, emb_all[:], ROPE, S, PI / 2, "ca",
                                            res_pool=rp, tagw="w")
                sin_own = _range_reduce_sin(nc, tp, emb_own[:], ROPE, SL, 0.0, "so")
                cos_own = _range_reduce_sin(nc, tp, emb_own[:], ROPE, SL, PI / 2, "co")

                # ---- kv wave: chunks c0..c3 + pe accumulate over all hc ----
                acc_kv = [accp.tile([128, SL], f32, name=f"acc_kv{c}", tag="acc", bufs=6)
                          for c in range(KCH)]
                acc_pe = accp.tile([128, SL], f32, name="acc_pe", tag="acc", bufs=6)
                for hc in range(HCH):
                    st = (hc == 0)
                    sp = (hc == HCH - 1)
                    for c in range(KCH):
                        nc.tensor.matmul(acc_kv[c][:], wkv_tiles[hc][:, 128 * c:128 * (c + 1)],
                                         xts[hc][:], start=st, stop=sp)
                    nc.tensor.matmul(acc_pe[0:ROPE, :], wkv_tiles[hc][:, KVLR:KVLR + ROPE],
                                     xts[hc][:], start=st, stop=sp)

                # kv ssq + rms scale
                ssq_kv = pp1.tile([1, SL], f32, name="ssq_kv", tag="ssq", bufs=1)
                sqs = []
                for c in range(KCH):
                    sq = tp.tile([128, SL], dt, name=f"sqk{c}", tag="sq", bufs=4)
                    nc.scalar.activation(sq[:], acc_kv[c][:], AF.Square)
                    sqs.append(sq)
                for c in range(KCH):
                    nc.tensor.matmul(ssq_kv[:], ocol[:], sqs[c][:],
                                     start=(c == 0), stop=(c == KCH - 1))
                ms_kv = tp.tile([1, SL], f32, name="ms_kv", tag="ms", bufs=2)
                nc.scalar.activation(ms_kv[:], ssq_kv[:], AF.Sqrt, scale=1.0 / KVLR)
                rkv = tp.tile([1, SL], f32, name="rkv", tag="rr", bufs=2)
                nc.vector.reciprocal_approx_fast(out=rkv[:], in_=ms_kv[:])
                rkvr = tp.tile([1, SL], f32r, name="rkvr", tag="rrr", bufs=2)
                with nc.allow_low_precision(reason="f32r rounding of rms scale"):
                    nc.vector.tensor_copy(rkvr[:], rkv[:])
                bc_kv = pp1.tile([128, SL], f32, name="bc_kv", tag="bc", bufs=1)
                nc.tensor.matmul(bc_kv[:], orow[:], rkvr[:], start=True, stop=True)
                bckv_sb = tp.tile([128, SL], f32, name="bckv_sb", tag="bc_sb", bufs=2)
                nc.scalar.activation(bckv_sb[:], bc_kv[:], AF.Copy)

                # k_pe rope
                krot = tp.tile([ROPE, SL], f32, name="krot", tag="krot", bufs=1)
                nc.vector.tensor_scalar(out=krot[0:32, :], in0=acc_pe[32:64, :],
                                        scalar1=-1.0, scalar2=None, op0=ALU.mult)
                nc.vector.tensor_copy(krot[32:64, :], acc_pe[0:32, :])
                kro = tp.tile([ROPE, SL], f32, name="kro", tag="kro", bufs=1)
                nc.vector.tensor_mul(kro[:], acc_pe[0:ROPE, :], cos_own[:])
                krs = tp.tile([ROPE, SL], f32, name="krs", tag="krs", bufs=1)
                nc.vector.tensor_mul(krs[:], krot[:], sin_own[:])
                kfin = tp.tile([ROPE, SL], dt, name="kfin", tag="kfin", bufs=1)
                nc.vector.tensor_add(kfin[:], kro[:], krs[:])
                nc.scalar.dma_start(out=ag_in_kv[KVLR:KVLR + ROPE, :], in_=kfin[:])

                for c in range(KCH):
                    sc = tp.tile([128, SL], dt, name=f"sck{c}", tag="sc", bufs=4)
                    nc.vector.tensor_mul(sc[:], acc_kv[c][:], bckv_sb[:])
                    nc.scalar.dma_start(out=ag_in_kv[128 * c:128 * (c + 1), :], in_=sc[:])

                nc.gpsimd.collective_compute(
                    "AllGather", ALU.bypass,
                    replica_groups=[list(range(NC_))],
                    ins=[ag_in_kv[:]], outs=[ag_out_kv[:]],
                )

                # ---- q wave in two halves of 6 chunks (PSUM bank budget) ----
                ssq_q = pp1.tile([1, SL], f32, name="ssq_q", tag="ssq", bufs=1)
                sqq = []
                raw_q = []
                acc_q1 = [accp.tile([128, SL], f32, name=f"acc_q1_{c}", tag="acc", bufs=6)
                          for c in range(QHALF)]
                for hc in range(HCH):
                    st = (hc == 0)
                    sp = (hc == HCH - 1)
                    for c in range(QHALF):
                        nc.tensor.matmul(acc_q1[c][:], wq_tiles[hc][:, 128 * c:128 * (c + 1)],
                                         xts[hc][:], start=st, stop=sp)
                for c in range(QHALF):
                    sq = tp.tile([128, SL], dt, name=f"sqq{c}", tag="sq", bufs=4)
                    nc.scalar.activation(sq[:], acc_q1[c][:], AF.Square)
                    sqq.append(sq)
                    raw = tp.tile([128, SL], dt, name=f"rawq{c}", tag=f"raw{c}", bufs=1)
                    nc.vector.tensor_copy(raw[:], acc_q1[c][:])
                    raw_q.append(raw)
                for c in range(QHALF):
                    nc.tensor.matmul(ssq_q[:], ocol[:], sqq[c][:],
                                     start=(c == 0), stop=False)

                acc_q2 = [accp.tile([128, SL], f32, name=f"acc_q2_{c}", tag="acc", bufs=6)
                          for c in range(QHALF)]
                for hc in range(HCH):
                    st = (hc == 0)
                    sp = (hc == HCH - 1)
                    for c in range(QHALF):
                        nc.tensor.matmul(acc_q2[c][:],
                                         wq_tiles[hc][:, 128 * (c + QHALF):128 * (c + QHALF + 1)],
                                         xts[hc][:], start=st, stop=sp)
                for c in range(QHALF):
                    sq = tp.tile([128, SL], dt, name=f"sqq{c + QHALF}", tag="sq", bufs=4)
                    nc.scalar.activation(sq[:], acc_q2[c][:], AF.Square)
                    sqq.append(sq)
                for c in range(QHALF):
                    nc.tensor.matmul(ssq_q[:], ocol[:], sqq[c + QHALF][:],
                                     start=False, stop=(c == QHALF - 1))
                ms_q = tp.tile([1, SL], f32, name="ms_q", tag="ms", bufs=2)
                nc.scalar.activation(ms_q[:], ssq_q[:], AF.Sqrt, scale=1.0 / QLR)
                rq = tp.tile([1, SL], f32, name="rq", tag="rr", bufs=2)
                nc.vector.reciprocal_approx_fast(out=rq[:], in_=ms_q[:])
                rqr = tp.tile([1, SL], f32r, name="rqr", tag="rrr", bufs=2)
                with nc.allow_low_precision(reason="f32r rounding of rms scale"):
                    nc.vector.tensor_copy(rqr[:], rq[:])
                bc_q = pp1.tile([128, SL], f32, name="bc_q", tag="bc", bufs=1)
                nc.tensor.matmul(bc_q[:], orow[:], rqr[:], start=True, stop=True)
                bcq_sb = tp.tile([128, SL], f32, name="bcq_sb", tag="bc_sb", bufs=2)
                nc.scalar.activation(bcq_sb[:], bc_q[:], AF.Copy)

                for c in range(QHALF):
                    sc = tp.tile([128, SL], dt, name=f"scq{c}", tag="sc", bufs=4)
                    nc.vector.tensor_mul(sc[:], raw_q[c][:], bcq_sb[:])
                    nc.scalar.dma_start(out=ag_in_qa[128 * c:128 * (c + 1), :], in_=sc[:])
                nc.gpsimd.collective_compute(
                    "AllGather", ALU.bypass,
                    replica_groups=[list(range(NC_))],
                    ins=[ag_in_qa[:]], outs=[ag_out_qa[:]],
                )
                for c in range(QHALF):
                    sc = tp.tile([128, SL], dt, name=f"scq{c + QHALF}", tag="sc", bufs=4)
                    nc.vector.tensor_mul(sc[:], acc_q2[c][:], bcq_sb[:])
                    nc.scalar.dma_start(out=ag_in_qb[128 * c:128 * (c + 1), :], in_=sc[:])
                nc.gpsimd.collective_compute(
                    "AllGather", ALU.bypass,
                    replica_groups=[list(range(NC_))],
                    ins=[ag_in_qb[:]], outs=[ag_out_qb[:]],
                )

            agkv_r = ag_out_kv.rearrange("(r c) q -> r c q", r=NC_)
            agqa_r = ag_out_qa.rearrange("(r c) q -> r c q", r=NC_)
            agqb_r = ag_out_qb.rearrange("(r c) q -> r c q", r=NC_)

            # ================= Stage B =================
            with tc.tile_pool(name="sb_res", bufs=1) as sbp, \
                 tc.tile_pool(name="sb_tmp", bufs=2) as tp, \
                 tc.tile_pool(name="sb_qa", bufs=2) as qap, \
                 tc.tile_pool(name="sb_pt", bufs=4) as ptp, \
                 tc.tile_pool(name="sb_mm", bufs=2, space="PSUM") as pmm, \
                 tc.tile_pool(name="sb_at", bufs=2, space="PSUM") as pat, \
                 tc.tile_pool(name="sb_ps1", bufs=1, space="PSUM") as pp1, \
                 tc.tile_pool(name="sb_wo", bufs=2, space="PSUM") as pwo:

                # K^T and V (both heads)
                kpe_g = sbp.tile([ROPE, S], dt, name="kpe_g")
                for r in range(NC_):
                    nc.sync.dma_start(out=kpe_g[:, SL * r:SL * (r + 1)],
                                      in_=agkv_r[r, KVLR:KVLR + ROPE, :])
                kT = [sbp.tile([128, S], dt, name=f"kT{h}") for h in range(HPC)]
                v_t = [sbp.tile([128, HPC * VD], dt, name=f"v_t{kb}") for kb in range(NKB)]
                with tc.tile_pool(name="sb_ckv", bufs=1) as ckvp:
                    ckv_g = []
                    for j in range(KCH):
                        t = ckvp.tile([128, S], dt, name=f"ckv_g{j}")
                        for r in range(NC_):
                            nc.sync.dma_start(out=t[:, SL * r:SL * (r + 1)],
                                              in_=agkv_r[r, 128 * j:128 * (j + 1), :])
                        ckv_g.append(t)
                    ei = 0
                    for h in range(HPC):
                        for kc in range(S // 512):
                            ps = pmm.tile([128, 512], f32, name=f"kt_ps{h}_{kc}", tag="mm", bufs=2)
                            for l in range(KCH):
                                nc.tensor.matmul(ps[:], wkk_t[l][:, NOPE * h:NOPE * (h + 1)],
                                                 ckv_g[l][:, 512 * kc:512 * (kc + 1)],
                                                 start=(l == 0), stop=(l == KCH - 1))
                            eng = nc.vector if ei % 2 == 0 else nc.scalar
                            if ei % 2 == 0:
                                eng.tensor_copy(kT[h][:, 512 * kc:512 * (kc + 1)], ps[:])
                            else:
                                eng.activation(kT[h][:, 512 * kc:512 * (kc + 1)], ps[:], AF.Copy)
                            ei += 1
                    for kb in range(NKB):
                        ps = pmm.tile([128, HPC * VD], f32, name=f"v_ps{kb}", tag="mm", bufs=2)
                        for l in range(KCH):
                            nc.tensor.matmul(ps[:], ckv_g[l][:, 128 * kb:128 * (kb + 1)], wkv_t[l][:],
                                             start=(l == 0), stop=(l == KCH - 1))
                        if kb % 2 == 0:
                            nc.vector.tensor_copy(v_t[kb][:], ps[:])
                        else:
                            nc.scalar.activation(v_t[kb][:], ps[:], AF.Copy)

                # ---- q projections: half a (chunks 0..5), then half b ----
                qa_pa = {}
                for p in range(NPANEL):
                    for l in range(QHALF):
                        t = qap.tile([128, PANEL], dt, name=f"qa_pa{p}_{l}", tag=f"qa_pa{l}", bufs=2)
                        for r in range(2):
                            nc.sync.dma_start(out=t[:, SL * r:SL * (r + 1)],
                                              in_=agqa_r[2 * p + r, 128 * l:128 * (l + 1), :])
                        qa_pa[(p, l)] = t
                qn_a = {}
                qr_a = {}
                for p in range(NPANEL):
                    for h in range(HPC):
                        hcol = 256 * h
                        ps_qn = pmm.tile([128, PANEL], f32, name=f"qna_ps{h}_{p}", tag="mm", bufs=2)
                        for l in range(QHALF):
                            nc.tensor.matmul(ps_qn[:], wqb_t[l][:, hcol:hcol + NOPE],
                                             qa_pa[(p, l)][:], start=(l == 0), stop=(l == QHALF - 1))
                        ps_qr = pmm.tile([128, PANEL], f32, name=f"qra_ps{h}_{p}", tag="mm", bufs=2)
                        for l in range(QHALF):
                            nc.tensor.matmul(ps_qr[:], wqb_t[l][:, hcol + NOPE:hcol + 256],
                                             qa_pa[(p, l)][:], start=(l == 0), stop=(l == QHALF - 1))
                        tn = sbp.tile([128, PANEL], f32, name=f"qn_a{h}_{p}")
                        nc.scalar.activation(tn[:], ps_qn[:], AF.Copy)
                        qn_a[(h, p)] = tn
                        tr_lo = sbp.tile([ROPE, PANEL], dt, name=f"qr_alo{h}_{p}")
                        nc.vector.tensor_copy(tr_lo[:], ps_qr[0:ROPE, :])
                        tr_hi = sbp.tile([ROPE, PANEL], dt, name=f"qr_ahi{h}_{p}")
                        nc.vector.tensor_copy(tr_hi[:], ps_qr[ROPE:2 * ROPE, :])
                        qr_a[(h, p)] = (tr_lo, tr_hi)

                qa_pb = {}
                for p in range(NPANEL):
                    for l in range(QHALF):
                        t = qap.tile([128, PANEL], dt, name=f"qa_pb{p}_{l}", tag=f"qa_pb{l}", bufs=2)
                        for r in range(2):
                            nc.sync.dma_start(out=t[:, SL * r:SL * (r + 1)],
                                              in_=agqb_r[2 * p + r, 128 * l:128 * (l + 1), :])
                        qa_pb[(p, l)] = t
                qn_sb = {}
                qp_sb = {}
                for p in range(NPANEL):
                    qs = slice(PANEL * p, PANEL * (p + 1))
                    for h in range(HPC):
                        hcol = 256 * h
                        ps_qn = pmm.tile([128, PANEL], f32, name=f"qnb_ps{h}_{p}", tag="mm", bufs=2)
                        for l in range(QHALF):
                            nc.tensor.matmul(ps_qn[:], wqb_t[l + QHALF][:, hcol:hcol + NOPE],
                                             qa_pb[(p, l)][:], start=(l == 0), stop=(l == QHALF - 1))
                        ps_qr = pmm.tile([128, PANEL], f32, name=f"qrb_ps{h}_{p}", tag="mm", bufs=2)
                        for l in range(QHALF):
                            nc.tensor.matmul(ps_qr[:], wqb_t[l + QHALF][:, hcol + NOPE:hcol + 256],
                                             qa_pb[(p, l)][:], start=(l == 0), stop=(l == QHALF - 1))
                        qn = sbp.tile([128, PANEL], dt, name=f"qn_sb{h}_{p}")
                        nc.vector.tensor_add(qn[:], qn_a[(h, p)][:], ps_qn[:])
                        qn_sb[(h, p)] = qn
                        tr_lo, tr_hi = qr_a[(h, p)]
                        s_lo = tp.tile([ROPE, PANEL], f32, name=f"slo{h}_{p}", tag="slo", bufs=2)
                        nc.vector.tensor_add(s_lo[:], tr_lo[:], ps_qr[0:ROPE, :])
                        s_hi = tp.tile([ROPE, PANEL], f32, name=f"shi{h}_{p}", tag="shi", bufs=2)
                        nc.vector.tensor_add(s_hi[:], tr_hi[:], ps_qr[ROPE:2 * ROPE, :])
                        qt1 = tp.tile([ROPE, PANEL], f32, name=f"qt1_{h}_{p}", tag="qt1", bufs=2)
                        nc.vector.tensor_mul(qt1[:], s_lo[:], cos_all[:, qs])
                        qt2 = tp.tile([ROPE, PANEL], f32, name=f"qt2_{h}_{p}", tag="qt2", bufs=2)
                        nc.vector.tensor_mul(qt2[:], s_hi[:], sin_all[:, qs])
                        qp = sbp.tile([ROPE, PANEL], dt, name=f"qp_sb{h}_{p}")
                        nc.vector.tensor_add(qp[:], qt1[:], qt2[:])
                        qp_sb[(h, p)] = qp

                # ---- attention + per-panel Wo partials + split ReduceScatter ----
                for p in range(NPANEL):
                    at_ps = {}
                    for h in range(HPC):
                        nkb = 4 * (p + 1)
                        ps_at = pat.tile([128, PANEL], f32, name=f"at_ps{h}_{p}", tag="at", bufs=2)
                        ps_sum = pp1.tile([1, PANEL], f32, name=f"sum_ps{h}_{p}", tag="sum", bufs=1)
                        pts = {}

                        def consume(kb, nkb=nkb, ps_at=ps_at, ps_sum=ps_sum, pts=pts, h=h):
                            nc.tensor.matmul(ps_at[:], v_t[kb][:, VD * h:VD * (h + 1)], pts[kb][:],
                                             start=(kb == 0), stop=(kb == nkb - 1))
                            nc.tensor.matmul(ps_sum[:], ocol[:], pts[kb][:],
                                             start=(kb == 0), stop=(kb == nkb - 1))

                        for kb in range(nkb):
                            ps_sc = pmm.tile([128, PANEL], f32, name=f"sc_ps{h}_{p}_{kb}",
                                             tag="mm", bufs=2)
                            nc.tensor.matmul(ps_sc[:], kT[h][:, 128 * kb:128 * (kb + 1)],
                                             qn_sb[(h, p)][:], start=True, stop=False)
                            nc.tensor.matmul(ps_sc[:], kpe_g[:, 128 * kb:128 * (kb + 1)],
                                             qp_sb[(h, p)][:], start=False, stop=True)
                            pt = ptp.tile([128, PANEL], dt, name=f"pt{h}_{p}_{kb}", tag="pt", bufs=4)
                            if kb >= 4 * p:
                                j = kb - 4 * p
                                msc = tp.tile([128, PANEL], f32, name=f"msc{h}_{p}_{kb}",
                                              tag="msc", bufs=2)
                                nc.vector.tensor_add(msc[:], ps_sc[:],
                                                     mask_sb[:, PANEL * j:PANEL * (j + 1)])
                                nc.scalar.activation(pt[:], msc[:], AF.Exp)
                            else:
                                nc.scalar.activation(pt[:], ps_sc[:], AF.Exp)
                            pts[kb] = pt
                            if kb > 0:
                                consume(kb - 1)
                        consume(nkb - 1)
                        rec = tp.tile([1, PANEL], f32, name=f"rec{h}_{p}", tag="rec", bufs=2)
                        nc.vector.reciprocal_approx_fast(out=rec[:], in_=ps_sum[:])
                        recr = tp.tile([1, PANEL], f32r, name=f"recr{h}_{p}", tag="recr", bufs=2)
                        with nc.allow_low_precision(reason="f32r rounding of softmax recip"):
                            nc.vector.tensor_copy(recr[:], rec[:])
                        bc = pp1.tile([128, PANEL], f32, name=f"bc_ps{h}_{p}", tag="bcb", bufs=1)
                        nc.tensor.matmul(bc[:], orow[:], recr[:], start=True, stop=True)
                        bc_sb = tp.tile([128, PANEL], f32, name=f"bc_sb{h}_{p}", tag="bc_sb", bufs=2)
                        nc.scalar.activation(bc_sb[:], bc[:], AF.Copy)
                        at_p = tp.tile([128, PANEL], dt, name=f"at_p{h}_{p}", tag="at_p", bufs=3)
                        nc.vector.tensor_mul(at_p[:], ps_at[:], bc_sb[:])
                        at_ps[h] = at_p

                    # Wo partial for this panel: out rows = panel's 4 seq blocks
                    g = p // 2
                    for sb in range(4):
                        ev = tp.tile([128, HID], RS_DT, name=f"woev{p}_{sb}", tag="woev", bufs=2)
                        for n in range(4):
                            ps_o = pwo.tile([128, 512], f32, name=f"wo_ps{p}_{sb}_{n}",
                                            tag="wo", bufs=2)
                            for h in range(HPC):
                                nc.tensor.matmul(ps_o[:], at_ps[h][:, 128 * sb:128 * (sb + 1)],
                                                 wo_sb[h][:, 512 * n:512 * (n + 1)],
                                                 start=(h == 0), stop=(h == HPC - 1))
                            if sb % 2 == 0:
                                nc.vector.tensor_copy(ev[:, 512 * n:512 * (n + 1)], ps_o[:])
                            else:
                                nc.scalar.activation(ev[:, 512 * n:512 * (n + 1)], ps_o[:], AF.Copy)
                        row0 = PANEL * (p % 2) + 128 * sb
                        if sb % 2 == 0:
                            nc.gpsimd.dma_start(out=rs_in[g][row0:row0 + 128, :], in_=ev[:])
                        else:
                            nc.scalar.dma_start(out=rs_in[g][row0:row0 + 128, :], in_=ev[:])
                    if p % 2 == 1:
                        nc.gpsimd.collective_compute(
                            "ReduceScatter", ALU.add,
                            replica_groups=[list(range(NC_))],
                            ins=[rs_in[g][:]], outs=[rs_out[g][:]],
                        )

                # ---- final: convert RS shards to f32 and store ----
                for g in range(2):
                    t = tp.tile([128, HID], RS_DT, name=f"ro_sb{g}", tag="ro", bufs=1)
                    nc.sync.dma_start(out=t[:], in_=rs_out[g][:])
                    o = tp.tile([128, HID], f32, name=f"o_f32_{g}", tag="of", bufs=1)
                    nc.scalar.activation(o[:], t[:], AF.Copy)
                    nc.sync.dma_start(out=out_loc[128 * g:128 * (g + 1), :], in_=o[:])

    nc.compile()
    return nc


def _to_dt(a, dt):
    if dt == bf16:
        return np.ascontiguousarray(a.astype(ml_dtypes.bfloat16))
    return np.ascontiguousarray(a.astype(np.float32))


def _prepare_inputs(dt, hidden_states, position_ids, Wqa, qa_ln_w, Wqb, Wkva, kv_ln_w, Wkvb, Wo):
    perm = np.concatenate([np.arange(0, ROPE, 2), np.arange(1, ROPE, 2)])
    X = np.asarray(hidden_states, np.float32).reshape(S, HID)
    pos_f = np.ascontiguousarray(np.asarray(position_ids, np.float32).reshape(1, S))
    Wqa = np.asarray(Wqa, np.float32)
    Wkva = np.asarray(Wkva, np.float32)
    wa_kv = np.concatenate([Wkva[:, :KVLR], Wkva[:, KVLR:][:, perm]], axis=1)  # (2048, 576)
    wqb_base = np.asarray(Wqb, np.float32) * np.asarray(qa_ln_w, np.float32)[:, None]
    wkvb_base = np.asarray(Wkvb, np.float32) * np.asarray(kv_ln_w, np.float32)[:, None]
    Wo = np.asarray(Wo, np.float32)

    head_blocks = []
    for h in range(NH):
        cols = wqb_base[:, 192 * h:192 * (h + 1)] * SM_SCALE
        nope = cols[:, :NOPE]
        pe_d = cols[:, NOPE:][:, perm]
        rot = np.concatenate([-pe_d[:, 32:], pe_d[:, :32]], axis=1)
        head_blocks.append(np.concatenate([nope, pe_d, rot], axis=1))  # (1536, 256)
    k_blocks = [wkvb_base[:, 256 * h:256 * h + NOPE] for h in range(NH)]
    v_blocks = [wkvb_base[:, 256 * h + NOPE:256 * (h + 1)] for h in range(NH)]

    inv = (1.0 / (THETA ** (np.arange(0, ROPE, 2, dtype=np.float32) / ROPE))).astype(np.float32)
    invf_np = np.concatenate([inv, inv])

    # diagonal masks: block j, mask[r, col] = NEG where col < 128 j + r
    cols = np.arange(PANEL)[None, :]
    rows = np.arange(128)[:, None]
    mask_np = np.concatenate(
        [np.where(cols < 128 * j + rows, NEG, 0.0) for j in range(4)], axis=1
    ).astype(np.float32)

    wa_kv_d = _to_dt(wa_kv, dt)
    wa_q_d = _to_dt(Wqa, dt)
    mask_d = _to_dt(mask_np, dt)
    ones_col_d = _to_dt(np.ones((128, 1), np.float32), dt)

    in_maps = []
    for c in range(NC_):
        rows_c = slice(SL * c, SL * (c + 1))
        in_maps.append({
            "x_t": _to_dt(X[rows_c, :].T, dt),
            "pos": np.ascontiguousarray(pos_f[:, rows_c]),
            "pos_all": pos_f,
            "wa_kv": wa_kv_d,
            "wa_q": wa_q_d,
            "wqb": _to_dt(np.concatenate([head_blocks[HPC * c + h] for h in range(HPC)], axis=1), dt),
            "wkvb_k": _to_dt(np.concatenate([k_blocks[HPC * c + h] for h in range(HPC)], axis=1), dt),
            "wkvb_v": _to_dt(np.concatenate([v_blocks[HPC * c + h] for h in range(HPC)], axis=1), dt),
            "wo_h": _to_dt(np.concatenate([Wo[VD * (HPC * c + h):VD * (HPC * c + h + 1), :]
                                           for h in range(HPC)], axis=0), dt),
            "mask": mask_d,
            "ones_col": ones_col_d,
            "ones_row": np.ones((1, 128), np.float32),
            "invf_col": invf_np.reshape(ROPE, 1).copy(),
        })
    return in_maps


def run(inputs, trace=False, trace_cores=None, dt=None):
    dt = dt if dt is not None else DT
    key = ("nc", str(dt))
    if key not in _CACHE:
        _CACHE[key] = build_program(dt)
    nc = _CACHE[key]
    in_maps = _prepare_inputs(dt, **inputs)
    res = run_bass_kernel_spmd(nc, in_maps, list(range(NC_)), trace=trace,
                               trace_cores=trace_cores)
    # reassemble: group g covers seq rows [1024 g, 1024 (g+1)); core c holds
    # rows [1024 g + 128 c, 1024 g + 128 (c+1)) in out_loc rows [128 g, 128 (g+1))
    out = np.empty((S, HID), np.float32)
    for c in range(NC_):
        o = res.results[c]["out_loc"]
        for g in range(2):
            out[1024 * g + 128 * c:1024 * g + 128 * (c + 1), :] = o[128 * g:128 * (g + 1), :]
    return out.reshape(1, S, HID), res


def kernel(**inputs) -> np.ndarray:
    out, _ = run(inputs, trace=False)
    return out
